# revision 1
# baseline (speedup 1.0000x reference)
"""MetaNet image-encoder kernel for 8 Trainium2 NeuronCores.

Sharding strategy (tensor parallel over H1 for the dominant dW1 einsum):
  - core m owns H1 columns [m*128, (m+1)*128) of W1/dW1/db1 -> computes its
    slice of the composed hidden vector h fully locally (the task-vector sum
    over t stays on-core).
  - core m owns F columns [m*64, (m+1)*64) of W2/dW2/db2/b2 -> computes its
    slice of the output.
  - batch samples {2m, 2m+1} are pooled on core m; pooled xp is AllGathered.
  - tiny collectives: AllGather xp (75KB/core), AllReduce base partials
    (32KB), AllGather h (8KB/core).

The host concatenates W1's slice with the 8 task-vector slices into one
[9, 9408, 128] stack; the kernel streams it as 3 groups of 3 matrices so
each matmul has a 384-wide moving operand and can use float32r (full PE
rate) instead of fp32 (quarter rate). Per-core HBM traffic ~47MB -> ridge.
"""

import numpy as np

B, T, DIN, H1, F, MH = 16, 8, 9408, 1024, 512, 128
NCORES = 8
SH = H1 // NCORES  # 128  H1 slice per core
SF = F // NCORES   # 64   F slice per core
BL = B // NCORES   # 2    batch samples pooled per core
KC = 112           # matmul contraction chunk (9408 = 84*112)
NK = DIN // KC     # 84
NG = 3             # matrix groups in the einsum stream (9 = 3*3)
GW = 3             # matrices per group
NPC = 6            # DMA pieces per group
PCH = NK // NPC    # 14 contraction chunks per DMA piece

_CACHE = {}


def _build(solo=False):
    import concourse.bass as bass
    import concourse.mybir as mybir
    import concourse.tile as tile
    from concourse.tile_rust import add_dep_helper
    from concourse import bacc
    from concourse.masks import make_identity

    fp32 = mybir.dt.float32
    f32r = mybir.dt.float32r
    Alu = mybir.AluOpType
    Act = mybir.ActivationFunctionType

    nc = bacc.Bacc("TRN2", target_bir_lowering=False, debug=False,
                   num_devices=1 if solo else NCORES)

    def inp(name, shape, dt=None):
        return nc.dram_tensor(name, shape, dt or fp32, kind="ExternalInput")

    x_loc = inp("x_loc", [BL, 3, 224, 224])
    Rpool = inp("Rpool", [224, 56])
    dWall = inp("dWall", [NG, DIN, GW * SH], f32r)  # [W1_s; dW1_s] interleaved
    b1_s = inp("b1_s", [1, SH])
    db1_s = inp("db1_s", [T, SH])
    W2_r = inp("W2_r", [SH, F])
    W2_c = inp("W2_c", [H1, SF], f32r)
    dW2_s = inp("dW2_s", [H1, T * SF], f32r)  # rows=H1, cols=(t,f) interleaved
    db2_s = inp("db2_s", [T, SF])
    b2_s = inp("b2_s", [1, SF])
    mW1 = inp("mW1", [F, MH])
    mb1e = inp("mb1e", [1, MH])      # mb1 + b2 @ mW1 (precomputed on host)
    mW2 = inp("mW2", [MH, T * 4])
    mb2 = inp("mb2", [1, T * 4])
    out_ext = nc.dram_tensor("out", [B, SF], fp32, kind="ExternalOutput")

    RG = [list(range(NCORES))]

    with tile.TileContext(nc) as tc:
        with (
            tc.tile_pool(name="dram", bufs=1, space="DRAM") as dram,
            tc.tile_pool(name="consts", bufs=1) as consts,
            tc.tile_pool(name="sb", bufs=1) as sb,
            tc.tile_pool(name="dw1p", bufs=4) as dw1p,
            tc.tile_pool(name="small", bufs=2) as small,
            tc.tile_pool(name="ps_tr", bufs=2, space="PSUM") as ps_tr,
            tc.tile_pool(name="ps_acc", bufs=1, space="PSUM") as ps_acc,
            tc.tile_pool(name="ps_misc", bufs=2, space="PSUM") as ps_misc,
            tc.tile_pool(name="ps_g", bufs=3, space="PSUM") as ps_g,
        ):
            # ---- constants ----
            id16 = consts.tile([16, 16], fp32)
            make_identity(nc, id16)
            ones1 = consts.tile([1, 16], fp32)
            nc.gpsimd.memset(ones1, 1.0)

            # =========== phase P: pool local samples -> xp_loc [BL, DIN] ====
            Rsb = consts.tile([KC, 2, 56], fp32)
            nc.sync.dma_start(Rsb, Rpool.rearrange("(n p) i -> p n i", p=KC))
            xt_sb = consts.tile([KC, 2 * 3 * 2, 224], fp32)
            for b in range(BL):
                for c in range(3):
                    nc.sync.dma_start(
                        xt_sb[:, (b * 3 + c) * 2:(b * 3 + c) * 2 + 2, :],
                        x_loc[b, c].rearrange("(n p) col -> p n col", p=KC))

            # ---- prefetch all small weights so the DMA queue never drains
            b1s_sb = consts.tile([1, SH], fp32)
            nc.sync.dma_start(b1s_sb, b1_s[:])
            W2r_sb = sb.tile([SH, F], fp32)
            nc.sync.dma_start(W2r_sb, W2_r[:])
            mW1_sb = sb.tile([MH, 4, MH], fp32)
            nc.sync.dma_start(mW1_sb, mW1.rearrange("(k p) h -> p k h", p=MH))
            mb1_sb = consts.tile([1, MH], fp32)
            nc.sync.dma_start(mb1_sb, mb1e[:])
            mW2_sb = consts.tile([MH, T * 4], fp32)
            nc.sync.dma_start(mW2_sb, mW2[:])
            mb2_sb = consts.tile([1, T * 4], fp32)
            nc.sync.dma_start(mb2_sb, mb2[:])
            db1s_sb = consts.tile([T, SH], fp32)
            nc.sync.dma_start(db1s_sb, db1_s[:])
            db2s_sb = consts.tile([T, SF], fp32)
            nc.sync.dma_start(db2s_sb, db2_s[:])
            b2s_sb = consts.tile([1, SF], fp32)
            nc.sync.dma_start(b2s_sb, b2_s[:])
            W2c_sb = sb.tile([MH, NCORES, SF], f32r)
            nc.sync.dma_start(W2c_sb, W2_c.rearrange("(k p) f -> p k f", p=MH))
            # free layout (k, t, f): 4 adjacent tasks form a 256-wide rhs
            dW2s_sb = sb.tile([MH, NCORES, T * SF], f32r)
            nc.sync.dma_start(dW2s_sb,
                              dW2_s.rearrange("(k p) w -> p k w", p=MH))

            xp_loc = dram.tile([BL, DIN], fp32)
            xp_loc_v = xp_loc.rearrange("b (c i j) -> b i c j", c=3, i=56, j=56)
            for b in range(BL):
                xp_st = small.tile([56, 3, 56], fp32, tag="pool56")
                for c in range(3):
                    ps1 = ps_tr.tile([56, 224], fp32, tag="tr", name="ps_pool")
                    for n in range(2):
                        nc.tensor.matmul(ps1, Rsb[:, n, :],
                                         xt_sb[:, (b * 3 + c) * 2 + n, :],
                                         start=(n == 0), stop=(n == 1))
                    nc.vector.tensor_copy(xp_st[:, c, :], ps1[:, 0::4])
                    for q in range(1, 4):
                        nc.vector.scalar_tensor_tensor(
                            xp_st[:, c, :], ps1[:, q::4], 1.0, xp_st[:, c, :],
                            Alu.mult, Alu.add)
                nc.gpsimd.dma_start(xp_loc_v[b], xp_st)

            # =========== AllGather xp -> xp_all [B, DIN] ====================
            if solo:
                xp_all = dram.tile([B, DIN], fp32)
                ag_inst = nc.gpsimd.dma_start(xp_all[0:BL], xp_loc)
            else:
                xp_all = dram.tile([B, DIN], fp32, addr_space="Shared")
                ag_inst = nc.gpsimd.collective_compute(
                    "AllGather", Alu.bypass, replica_groups=RG,
                    ins=[xp_loc.opt()], outs=[xp_all.opt()])

            # =========== transpose xp -> xpT chunks [KC, NK*16] =============
            xp_sb = sb.tile([B, DIN], fp32)
            nc.gpsimd.dma_start(xp_sb, xp_all)
            xpT = sb.tile([KC, NK * 16], f32r)
            for n8 in range(0, NK, 8):
                w = min(8, NK - n8)
                ptr = ps_tr.tile([KC, 8 * 16], fp32, tag="tr", name="ps_xpt")
                for j in range(w):
                    nc.tensor.transpose(
                        ptr[:, j * 16:(j + 1) * 16],
                        xp_sb[:, (n8 + j) * KC:(n8 + j + 1) * KC], id16)
                nc.vector.tensor_copy(xpT[:, n8 * 16:(n8 + w) * 16],
                                      ptr[:, 0:w * 16])

            # =========== phase E: stream dWall, 3 groups of 3 matrices ======
            # group g holds matrices [3g, 3g+1, 3g+2] of [W1; dW1(0..7)];
            # psum [16, 384] accumulates xp @ each over the 84 K-chunks.
            htv = sb.tile([16, T * SH], fp32)
            pre1_sb = small.tile([16, SH], fp32, tag="pre1s", bufs=1)

            def einsum_group(g):
                pg = ps_g.tile([16, GW * SH], fp32, tag="g", name=f"ps_g{g}")
                for q in range(NPC):
                    dwt = dw1p.tile([KC, PCH, GW * SH], f32r, tag="dw",
                                    name=f"dw_{g}_{q}")
                    dma_inst = nc.sync.dma_start(
                        dwt,
                        dWall[g].rearrange("(n p) w -> p n w", p=KC)
                        [:, q * PCH:(q + 1) * PCH])
                    if g == 0 and q == 0:
                        add_dep_helper(dma_inst.ins, ag_inst.ins, sync=True,
                                       reason="stream after xp AllGather")
                    for u in range(PCH):
                        n = q * PCH + u
                        nc.tensor.matmul(pg,
                                         xpT[:, n * 16:(n + 1) * 16],
                                         dwt[:, u],
                                         start=(n == 0), stop=(n == NK - 1))
                if g == 0:
                    nc.vector.tensor_copy(pre1_sb, pg[:, 0:SH])
                    nc.vector.tensor_copy(htv[:, 0:2 * SH], pg[:, SH:3 * SH])
                else:
                    nc.vector.tensor_copy(
                        htv[:, (GW * g - 1) * SH:(GW * (g + 1) - 1) * SH], pg)

            einsum_group(0)

            # =========== phase C: base partial, AllReduce, metanet ==========
            b1bc_ps = ps_misc.tile([16, SH], fp32, tag="misc")
            nc.tensor.matmul(b1bc_ps, ones1, b1s_sb, start=True, stop=True)
            pre1b = small.tile([16, SH], fp32, tag="pre1b", bufs=1)
            nc.vector.scalar_tensor_tensor(
                pre1b, b1bc_ps, 1.0, pre1_sb, Alu.mult, Alu.add)

            basehid = small.tile([16, SH], fp32, tag="sm128")
            nc.scalar.activation(basehid, pre1b, Act.Relu)
            ptr = ps_tr.tile([SH, 16], fp32, tag="tr", name="ps_bht")
            nc.tensor.transpose(ptr, basehid, id16)
            bhT = small.tile([SH, 16], fp32, tag="smT")
            nc.vector.tensor_copy(bhT, ptr)

            bp_ps = ps_misc.tile([16, F], fp32, tag="misc")
            nc.tensor.matmul(bp_ps, bhT, W2r_sb, start=True, stop=True)
            bp_sb = small.tile([16, F], fp32, tag="sm512")
            nc.vector.tensor_copy(bp_sb, bp_ps)
            bp_in = dram.tile([16, F], fp32)
            nc.gpsimd.dma_start(bp_in, bp_sb)
            if solo:
                bp_out = dram.tile([16, F], fp32)
                nc.gpsimd.dma_start(bp_out[:], bp_in)
            else:
                bp_out = dram.tile([16, F], fp32, addr_space="Shared")
                nc.gpsimd.collective_compute(
                    "AllReduce", Alu.add, replica_groups=RG,
                    ins=[bp_in.opt()], outs=[bp_out.opt()])
            base_sb = small.tile([16, F], fp32, tag="sm512")
            nc.gpsimd.dma_start(base_sb, bp_out)

            for _g in range(1, NG):
                einsum_group(_g)

            baseT = small.tile([MH, 4 * 16], fp32, tag="smT")
            ptrk = ps_tr.tile([MH, 4 * 16], fp32, tag="tr", name="ps_bT")
            for k in range(4):
                nc.tensor.transpose(ptrk[:, k * 16:(k + 1) * 16],
                                    base_sb[:, k * MH:(k + 1) * MH], id16)
            nc.vector.tensor_copy(baseT, ptrk)
            m1_ps = ps_misc.tile([16, MH], fp32, tag="misc")
            for k in range(4):
                nc.tensor.matmul(m1_ps, baseT[:, k * 16:(k + 1) * 16],
                                 mW1_sb[:, k, :], start=(k == 0), stop=False)
            nc.tensor.matmul(m1_ps, ones1, mb1_sb, start=False, stop=True)
            m1_sb = small.tile([16, MH], fp32, tag="sm128")
            nc.scalar.activation(m1_sb, m1_ps, Act.Relu)
            ptr = ps_tr.tile([MH, 16], fp32, tag="tr", name="ps_m1T")
            nc.tensor.transpose(ptr, m1_sb, id16)
            m1T = small.tile([MH, 16], fp32, tag="smT")
            nc.vector.tensor_copy(m1T, ptr)
            cf_ps = ps_misc.tile([16, T * 4], fp32, tag="misc")
            nc.tensor.matmul(cf_ps, m1T, mW2_sb, start=True, stop=False)
            nc.tensor.matmul(cf_ps, ones1, mb2_sb, start=False, stop=True)
            coefs = consts.tile([16, T * 4], fp32)
            nc.vector.tensor_copy(coefs, cf_ps)

            # cb1 term: transpose coefs[:,1::4] -> [T,16]; psum = cb1T @ db1_s
            ptr = ps_tr.tile([T, 16], fp32, tag="tr", name="ps_cb1")
            nc.tensor.transpose(ptr, coefs[:, 1::4], id16)
            cb1T = small.tile([T, 16], fp32, tag="smT")
            nc.vector.tensor_copy(cb1T, ptr)
            db1_ps = ps_acc.tile([16, SH], fp32, tag="db1")
            nc.tensor.matmul(db1_ps, cb1T, db1s_sb, start=True, stop=True)

            # weighted combine -> h = relu(pre1b + sum_t cW1_t*htv_t + db1t)
            h_pre = small.tile([16, SH], fp32, tag="sm128")
            nc.vector.scalar_tensor_tensor(
                h_pre, htv[:, 0:SH], coefs[:, 0:1], pre1b, Alu.mult, Alu.add)
            for t in range(1, T):
                nc.vector.scalar_tensor_tensor(
                    h_pre, htv[:, t * SH:(t + 1) * SH],
                    coefs[:, 4 * t:4 * t + 1], h_pre, Alu.mult, Alu.add)
            h_sb = small.tile([16, SH], fp32, tag="sm128")
            nc.vector.scalar_tensor_tensor(
                h_sb, db1_ps, 0.0, h_pre, Alu.add, Alu.add)
            nc.vector.tensor_scalar_max(h_sb, h_sb, 0.0)

            # ====== transpose h locally, AllGather the transposed slices ====
            ptrh = ps_tr.tile([SH, 16], fp32, tag="tr", name="ps_hT")
            nc.tensor.transpose(ptrh, h_sb, id16)
            hT_sb1 = small.tile([SH, 16], f32r, tag="smTr")
            nc.vector.tensor_copy(hT_sb1, ptrh)
            hT_loc = dram.tile([SH, 16], f32r)
            nc.gpsimd.dma_start(hT_loc, hT_sb1)
            if solo:
                hT_all = dram.tile([NCORES, SH, 16], f32r)
                nc.gpsimd.dma_start(hT_all[0], hT_loc)
            else:
                hT_all = dram.tile([NCORES, SH, 16], f32r, addr_space="Shared")
                nc.gpsimd.collective_compute(
                    "AllGather", Alu.bypass, replica_groups=RG,
                    ins=[hT_loc.opt()], outs=[hT_all.opt()])
            hT = sb.tile([SH, NCORES * 16], f32r)
            nc.gpsimd.dma_start(
                hT.rearrange("p (m b) -> p m b", m=NCORES),
                hT_all.rearrange("m p b -> p m b"))

            # =========== phase O: layer 2 on F slice ========================
            o_ps = ps_acc.tile([16, SF], fp32, tag="db1", name="o_ps")
            for k in range(NCORES):
                nc.tensor.matmul(o_ps, hT[:, k * 16:(k + 1) * 16],
                                 W2c_sb[:, k, :],
                                 start=(k == 0), stop=(k == NCORES - 1))
            otv = small.tile([16, T * SF], fp32, tag="otv")
            for g in range(2):
                og_ps = ps_g.tile([16, 4 * SF], fp32, tag="g",
                                  name=f"ps_og{g}")
                for k in range(NCORES):
                    nc.tensor.matmul(og_ps,
                                     hT[:, k * 16:(k + 1) * 16],
                                     dW2s_sb[:, k, g * 4 * SF:(g + 1) * 4 * SF],
                                     start=(k == 0), stop=(k == NCORES - 1))
                nc.vector.tensor_copy(
                    otv[:, g * 4 * SF:(g + 1) * 4 * SF], og_ps)

            ptr = ps_tr.tile([T, 16], fp32, tag="tr", name="ps_cb2")
            nc.tensor.transpose(ptr, coefs[:, 3::4], id16)
            cb2T = small.tile([T, 16], fp32, tag="smT")
            nc.vector.tensor_copy(cb2T, ptr)
            pb_ps = ps_misc.tile([16, SF], fp32, tag="misc")
            nc.tensor.matmul(pb_ps, cb2T, db2s_sb, start=True, stop=False)
            nc.tensor.matmul(pb_ps, ones1, b2s_sb, start=False, stop=True)

            out_acc = small.tile([16, SF], fp32, tag="smout")
            nc.vector.scalar_tensor_tensor(
                out_acc, otv[:, 0:SF], coefs[:, 2:3], o_ps, Alu.mult, Alu.add)
            for t in range(1, T):
                nc.vector.scalar_tensor_tensor(
                    out_acc, otv[:, t * SF:(t + 1) * SF],
                    coefs[:, 4 * t + 2:4 * t + 3], out_acc,
                    Alu.mult, Alu.add)
            out_fin = small.tile([16, SF], fp32, tag="smout")
            nc.vector.tensor_add(out_fin, out_acc, pb_ps)
            nc.sync.dma_start(out_ext[:], out_fin)

    nc.compile()
    return nc


def _get_nc():
    if "nc" not in _CACHE:
        _CACHE["nc"] = _build()
    return _CACHE["nc"]


def _make_rpool():
    R = np.zeros((224, 56), dtype=np.float32)
    for r in range(224):
        R[r, r // 4] = 1.0 / 16.0
    return R


def make_in_maps(x, W1, b1, W2, b2, dW1, db1, dW2, db2, mW1, mb1, mW2, mb2):
    f = lambda a: np.ascontiguousarray(a, dtype=np.float32)
    R = _make_rpool()
    mb1e = (mb1.astype(np.float64) + b2.astype(np.float64)
            @ mW1.astype(np.float64)).astype(np.float32)[None]
    in_maps = []
    for m in range(NCORES):
        sh = slice(m * SH, (m + 1) * SH)
        sf = slice(m * SF, (m + 1) * SF)
        dwall = np.concatenate([W1[None, :, sh], dW1[:, :, sh]], axis=0)
        dwall = np.ascontiguousarray(
            dwall.reshape(NG, GW, DIN, SH).transpose(0, 2, 1, 3)
            .reshape(NG, DIN, GW * SH))
        in_maps.append({
            "x_loc": f(x[m * BL:(m + 1) * BL]),
            "Rpool": R,
            "dWall": f(dwall),
            "b1_s": f(b1[None, sh]),
            "db1_s": f(db1[:, sh]),
            "W2_r": f(W2[sh, :]),
            "W2_c": f(W2[:, sf]),
            "dW2_s": f(dW2[:, :, sf].transpose(1, 0, 2).reshape(H1, T * SF)),
            "db2_s": f(db2[:, sf]),
            "b2_s": f(b2[None, sf]),
            "mW1": f(mW1),
            "mb1e": mb1e,
            "mW2": f(mW2),
            "mb2": f(mb2[None]),
        })
    return in_maps


def run_spmd(in_maps, trace=False, **kw):
    from concourse.bass_utils import run_bass_kernel_spmd
    nc = _get_nc()
    return run_bass_kernel_spmd(nc, in_maps, core_ids=list(range(NCORES)),
                                trace=trace, **kw)


def kernel(**inputs):
    inputs = {k: np.asarray(v, dtype=np.float32) for k, v in inputs.items()}
    in_maps = make_in_maps(**inputs)
    res = run_spmd(in_maps)
    out = np.concatenate([res.results[m]["out"] for m in range(NCORES)],
                         axis=1)
    return out.astype(np.float32)

